# revision 6
# baseline (speedup 1.0000x reference)
"""Trainium2 Bass kernel for nn_MaxRetrievalModel (sparse attention retrieval).

Math: the reference computes
    q = x_query[..,None]@Wq + bq                 [B,1,D]
    k = x_items@Wk + bk ; v = x_items@Wv + bv    [B,N,D]
    scores = (q.k)/sqrt(D) ; attn = sparsemax(scores)
    out = (attn@v)[:,0]@Wphi + bphi              [B,C]

Two exact algebraic identities collapse the 137-GFLOP K/V projections:
  * sparsemax is shift-invariant, so the bk term (constant over items)
    vanishes, and scores = x_items @ (s*Wk@q[b]) — a matvec per batch.
  * sparsemax output sums to exactly 1, so attn@v = (attn@x_items)@Wv + bv.
The sparsemax threshold tau solves sum(relu(z - tau)) = 1; Newton's method
on that piecewise-linear equation (monotone from tau0 = max(z) - 1)
converges exactly in <= ~9 steps for this regime, avoiding any sort.

Device work per core (4 batches): stream x shard (33.5 MB) once into SBUF,
per-partition dot products on DVE for scores, Newton on ACT/DVE/PE,
attn@x on PE, then the small Wv/Wphi tail on PE.

Sharding: data-parallel over batch B=32 across 8 cores (4 batches/core).
"""

import sys

if "/opt/trn_rl_repo" not in sys.path:
    sys.path.insert(0, "/opt/trn_rl_repo")

import numpy as np

B, N, D_IN, D, C = 32, 4096, 512, 512, 1000
NCORES = 8
BPC = B // NCORES          # batches per core
NCHUNK = N // 128          # 32 item chunks of 128
NIT = 14                   # Newton iterations (converges in <= 9 on this regime)

_CACHE = {}


def build_bass(repeat=1):
    """Build (and bacc-compile) the per-core Bass module.

    repeat > 1 replays the whole body; used by the test harness for
    differential hardware timing.
    """
    import concourse.bacc as bacc
    import concourse.tile as tile
    from concourse import mybir
    from contextlib import ExitStack

    f32 = mybir.dt.float32
    AF = mybir.ActivationFunctionType
    OP = mybir.AluOpType
    AX = mybir.AxisListType

    nc = bacc.Bacc("TRN2", target_bir_lowering=False, debug=False,
                   num_devices=NCORES)

    x = nc.declare_dram_parameter("x", [BPC, N, D_IN], f32, isOutput=False)
    kq = nc.declare_dram_parameter("kq", [128, BPC, D_IN], f32, isOutput=False)
    wv = nc.declare_dram_parameter("wv", [128, 4, D], f32, isOutput=False)
    wphi = nc.declare_dram_parameter("wphi", [128, 4, C], f32, isOutput=False)
    bvr = nc.declare_dram_parameter("bvr", [128, 4], f32, isOutput=False)
    bphir = nc.declare_dram_parameter("bphir", [BPC, C], f32, isOutput=False)
    consts = nc.declare_dram_parameter("consts", [128, 256], f32, isOutput=False)
    out = nc.declare_dram_parameter("out", [BPC, C], f32, isOutput=True)

    with tile.TileContext(nc) as tc, ExitStack() as ctx:
        cpool = ctx.enter_context(tc.tile_pool(name="consts", bufs=1))
        xpool = ctx.enter_context(tc.tile_pool(name="xb", bufs=2))
        zpool = ctx.enter_context(tc.tile_pool(name="zsc", bufs=2))
        ppool = ctx.enter_context(tc.tile_pool(name="prod", bufs=2))
        npool = ctx.enter_context(tc.tile_pool(name="newton", bufs=4))
        apool = ctx.enter_context(tc.tile_pool(name="attn", bufs=2))
        tpool = ctx.enter_context(tc.tile_pool(name="tail", bufs=2))
        pspool = ctx.enter_context(tc.tile_pool(name="ps", bufs=3, space="PSUM"))
        psupool = ctx.enter_context(tc.tile_pool(name="psu", bufs=2, space="PSUM"))

        cst = cpool.tile([128, 256], f32)
        nc.sync.dma_start(out=cst, in_=consts[:, :])
        ident = cst[:, 0:128]
        ones = cst[:, 128:256]

        kq_sb = cpool.tile([128, BPC, D_IN], f32)
        nc.sync.dma_start(out=kq_sb, in_=kq[:, :, :])
        wv_sb = cpool.tile([128, 4, D], f32)
        nc.sync.dma_start(out=wv_sb, in_=wv[:, :, :])
        wphi_sb = cpool.tile([128, 4, C], f32)
        nc.sync.dma_start(out=wphi_sb, in_=wphi[:, :, :])
        bvr_sb = cpool.tile([128, 4], f32)
        nc.sync.dma_start(out=bvr_sb, in_=bvr[:, :])
        bphir_sb = cpool.tile([BPC, C], f32)
        nc.sync.dma_start(out=bphir_sb, in_=bphir[:, :])

        for _rep in range(repeat):
            u_sbs = []

            for b in range(BPC):
                xb = xpool.tile([128, NCHUNK, D_IN], f32, tag="xb")
                xsrc = x[b].rearrange("(c p) d -> p c d", p=128)
                for i in range(4):
                    nc.sync.dma_start(out=xb[:, 8 * i:8 * (i + 1), :],
                                      in_=xsrc[:, 8 * i:8 * (i + 1), :])

                # scores z[p, c] = x[n=c*128+p, :] . kq   (DVE dot per partition)
                z_sc = zpool.tile([128, NCHUNK], f32, tag="zsc")
                for c in range(NCHUNK):
                    prod = ppool.tile([128, D_IN], f32, tag="prod")
                    nc.vector.scalar_tensor_tensor(
                        out=prod, in0=xb[:, c, :], scalar=1.0,
                        in1=kq_sb[:, b, :],
                        op0=OP.mult, op1=OP.mult,
                        accum_out=z_sc[:, c:c + 1])

                # tau0 = max(z) - 1, maintained negated+replicated [128,1]
                mx = npool.tile([128, 1], f32, tag="mx")
                nc.vector.tensor_reduce(out=mx, in_=z_sc, axis=AX.X, op=OP.max)
                ps_t = pspool.tile([1, 128], f32, tag="ps_small")
                nc.tensor.transpose(ps_t, mx, ident)
                vmax = npool.tile([1, 1], f32, tag="vmax")
                nc.vector.tensor_reduce(out=vmax, in_=ps_t, axis=AX.X, op=OP.max)
                ps_bc = pspool.tile([128, 1], f32, tag="ps_small")
                nc.tensor.matmul(ps_bc, ones[0:1, :], vmax, start=True, stop=True)
                negtau = npool.tile([128, 1], f32, tag="negtau")
                nc.vector.tensor_scalar(out=negtau, in0=ps_bc,
                                        scalar1=-1.0, scalar2=1.0,
                                        op0=OP.mult, op1=OP.add)

                # Newton: tau += (S(tau)-1)/count(tau)
                for _it in range(NIT):
                    SC = npool.tile([128, 2], f32, tag="SC")
                    relu_s = ppool.tile([128, NCHUNK], f32, tag="relus")
                    nc.scalar.activation(out=relu_s, in_=z_sc, func=AF.Relu,
                                         bias=negtau, scale=1.0,
                                         accum_out=SC[:, 0:1])
                    gt_s = ppool.tile([128, NCHUNK], f32, tag="gts")
                    nc.vector.tensor_scalar(out=gt_s, in0=relu_s,
                                            scalar1=0.0, scalar2=None,
                                            op0=OP.is_gt, op1=OP.add,
                                            accum_out=SC[:, 1:2])
                    ps_sc = pspool.tile([128, 2], f32, tag="ps_small")
                    nc.tensor.matmul(ps_sc, ones, SC, start=True, stop=True)
                    rcp = npool.tile([128, 1], f32, tag="rcp")
                    nc.vector.reciprocal(out=rcp, in_=ps_sc[:, 1:2])
                    delta = npool.tile([128, 1], f32, tag="delta")
                    nc.vector.scalar_tensor_tensor(
                        out=delta, in0=ps_sc[:, 0:1], scalar=-1.0, in1=rcp,
                        op0=OP.add, op1=OP.mult)
                    negtau2 = npool.tile([128, 1], f32, tag="negtau")
                    nc.vector.scalar_tensor_tensor(
                        out=negtau2, in0=delta, scalar=-1.0, in1=negtau,
                        op0=OP.mult, op1=OP.add)
                    negtau = negtau2

                attn_t = apool.tile([128, NCHUNK], f32, tag="attn")
                nc.scalar.activation(out=attn_t, in_=z_sc, func=AF.Relu,
                                     bias=negtau, scale=1.0)

                # u = attn @ x  -> [1, 512] accumulated over item chunks
                ps_u = psupool.tile([1, D_IN], f32, tag="psu")
                for c in range(NCHUNK):
                    nc.tensor.matmul(ps_u, attn_t[:, c:c + 1], xb[:, c, :],
                                     start=(c == 0), stop=(c == NCHUNK - 1))
                u_sb = tpool.tile([1, D_IN], f32, tag=f"usb{b}")
                nc.vector.tensor_copy(out=u_sb, in_=ps_u)
                u_sbs.append(u_sb)

            # tail: z = U@Wv + bv ; out = z@Wphi + bphi
            uT = tpool.tile([128, 4, BPC], f32, tag="uT")
            for ic in range(4):
                for b in range(BPC):
                    ps_tt = pspool.tile([128, 1], f32, tag="ps_small")
                    nc.tensor.transpose(ps_tt,
                                        u_sbs[b][0:1, ic * 128:(ic + 1) * 128],
                                        ident[0:1, 0:1])
                    nc.vector.tensor_copy(out=uT[:, ic, b:b + 1], in_=ps_tt)

            zT = tpool.tile([128, 4, BPC], f32, tag="zT")
            for jc in range(4):
                ps_z = pspool.tile([128, BPC], f32, tag="ps_small")
                for ic in range(4):
                    nc.tensor.matmul(ps_z, wv_sb[:, ic, jc * 128:(jc + 1) * 128],
                                     uT[:, ic, :], start=(ic == 0), stop=(ic == 3))
                nc.vector.tensor_scalar(out=zT[:, jc, :], in0=ps_z,
                                        scalar1=bvr_sb[:, jc:jc + 1], scalar2=None,
                                        op0=OP.add)

            out_sb = tpool.tile([BPC, C], f32, tag="out_sb")
            for h in range(2):
                ps_o = psupool.tile([BPC, C // 2], f32, tag="pso")
                for jc in range(4):
                    nc.tensor.matmul(ps_o, zT[:, jc, :],
                                     wphi_sb[:, jc, (C // 2) * h:(C // 2) * (h + 1)],
                                     start=(jc == 0), stop=(jc == 3))
                nc.vector.tensor_add(out_sb[:, (C // 2) * h:(C // 2) * (h + 1)],
                                     ps_o, bphir_sb[:, (C // 2) * h:(C // 2) * (h + 1)])
            nc.sync.dma_start(out=out[:, :], in_=out_sb)

    nc.compile()
    return nc


def host_prep(inputs):
    """Host-side O(B*D) prep: fold q/Wk/scale into per-batch kq vectors and
    pre-layout the small weight tensors for SBUF-friendly DMA."""
    f = lambda k: np.ascontiguousarray(np.asarray(inputs[k], dtype=np.float32))
    x_items, x_query = f("x_items"), f("x_query")
    Wq, bq, Wk = f("Wq"), f("bq"), f("Wk")
    Wv, bv, Wphi, bphi = f("Wv"), f("bv"), f("Wphi"), f("bphi")

    s = np.float32(D ** -0.5)
    Q = (x_query @ Wq + bq).astype(np.float32)            # [B, D]
    KQ = ((Q @ Wk.T) * s).astype(np.float32)              # [B, D_IN]

    wv_t = np.ascontiguousarray(Wv.reshape(4, 128, D).transpose(1, 0, 2))
    wphi_t = np.ascontiguousarray(Wphi.reshape(4, 128, C).transpose(1, 0, 2))
    bvr = np.ascontiguousarray(bv.reshape(4, 128).T)
    bphir = np.ascontiguousarray(np.broadcast_to(bphi, (BPC, C)))
    consts = np.concatenate([np.eye(128, dtype=np.float32),
                             np.ones((128, 128), np.float32)], axis=1)
    consts = np.ascontiguousarray(consts)

    in_maps = []
    for core in range(NCORES):
        sl = slice(core * BPC, (core + 1) * BPC)
        kq_c = np.ascontiguousarray(
            np.broadcast_to(KQ[sl][:, None, :], (BPC, 128, D_IN))
            .transpose(1, 0, 2))                          # [128, BPC, D_IN]
        in_maps.append({
            "x": np.ascontiguousarray(x_items[sl]),
            "kq": kq_c,
            "wv": wv_t,
            "wphi": wphi_t,
            "bvr": bvr,
            "bphir": bphir,
            "consts": consts,
        })
    return in_maps


def kernel(**inputs):
    from concourse.bass_utils import run_bass_kernel_spmd

    if "nc" not in _CACHE:
        _CACHE["nc"] = build_bass()
    nc = _CACHE["nc"]

    in_maps = host_prep(inputs)
    res = run_bass_kernel_spmd(nc, in_maps, list(range(NCORES)))
    return np.concatenate([res.results[c]["out"] for c in range(NCORES)],
                          axis=0).astype(np.float32)


# revision 9
# speedup vs baseline: 164.7371x; 164.7371x over previous
"""Trainium2 Bass kernel for nn_MaxRetrievalModel (sparse attention retrieval).

Math: the reference computes
    q = x_query[..,None]@Wq + bq                 [B,1,D]
    k = x_items@Wk + bk ; v = x_items@Wv + bv    [B,N,D]
    scores = (q.k)/sqrt(D) ; attn = sparsemax(scores)
    out = (attn@v)[:,0]@Wphi + bphi              [B,C]

Two exact algebraic identities collapse the 137-GFLOP K/V projections:
  * sparsemax is shift-invariant, so the bk term (constant over items)
    vanishes, and scores = x_items @ (s*Wk@q[b]) — a matvec per batch.
  * sparsemax output sums to exactly 1, so attn@v = (attn@x_items)@Wv + bv.
The sparsemax threshold tau solves sum(relu(z - tau)) = 1; Newton's method
on that piecewise-linear equation (monotone from tau0 = max(z) - 1)
converges exactly in <= ~9 steps for this regime, avoiding any sort.

Device work per core (4 batches): stream x shard (33.5 MB) once into SBUF,
per-partition dot products on DVE for scores, Newton on ACT/DVE/PE,
attn@x on PE, then the small Wv/Wphi tail on PE.

Sharding: data-parallel over batch B=32 across 8 cores (4 batches/core).
"""

import sys

if "/opt/trn_rl_repo" not in sys.path:
    sys.path.insert(0, "/opt/trn_rl_repo")

import numpy as np

B, N, D_IN, D, C = 32, 4096, 512, 512, 1000
NCORES = 8
BPC = B // NCORES          # batches per core
NCHUNK = N // 128          # 32 item chunks of 128
NIT = 14                   # Newton iterations (converges in <= 9 on this regime)

_CACHE = {}


def build_bass(repeat=1, nit=NIT, do_scores=True, do_newton=True, do_u=True,
               do_tail=True):
    """Build (and bacc-compile) the per-core Bass module.

    repeat > 1 replays the whole body (differential hardware timing);
    the do_* flags ablate stages for cost attribution. Correct output
    requires all stages on.
    """
    import concourse.bacc as bacc
    import concourse.tile as tile
    from concourse import mybir
    from contextlib import ExitStack

    f32 = mybir.dt.float32
    AF = mybir.ActivationFunctionType
    OP = mybir.AluOpType
    AX = mybir.AxisListType

    nc = bacc.Bacc("TRN2", target_bir_lowering=False, debug=False,
                   num_devices=NCORES)

    x = nc.declare_dram_parameter("x", [BPC, N, D_IN], f32, isOutput=False)
    kq = nc.declare_dram_parameter("kq", [128, BPC, D_IN], f32, isOutput=False)
    wv = nc.declare_dram_parameter("wv", [128, 4, D], f32, isOutput=False)
    wphi = nc.declare_dram_parameter("wphi", [128, 4, C], f32, isOutput=False)
    bvr = nc.declare_dram_parameter("bvr", [128, 4], f32, isOutput=False)
    bphir = nc.declare_dram_parameter("bphir", [BPC, C], f32, isOutput=False)
    consts = nc.declare_dram_parameter("consts", [128, 256], f32, isOutput=False)
    out = nc.declare_dram_parameter("out", [BPC, C], f32, isOutput=True)

    with tile.TileContext(nc) as tc, ExitStack() as ctx:
        cpool = ctx.enter_context(tc.tile_pool(name="consts", bufs=1))
        xpool = ctx.enter_context(tc.tile_pool(name="xb", bufs=2))
        zpool = ctx.enter_context(tc.tile_pool(name="zsc", bufs=2))
        ppool = ctx.enter_context(tc.tile_pool(name="prod", bufs=2))
        npool = ctx.enter_context(tc.tile_pool(name="newton", bufs=4))
        apool = ctx.enter_context(tc.tile_pool(name="attn", bufs=2))
        tpool = ctx.enter_context(tc.tile_pool(name="tail", bufs=2))
        pspool = ctx.enter_context(tc.tile_pool(name="ps", bufs=3, space="PSUM"))
        psupool = ctx.enter_context(tc.tile_pool(name="psu", bufs=2, space="PSUM"))

        cst = cpool.tile([128, 256], f32)
        nc.sync.dma_start(out=cst, in_=consts[:, :])
        ident = cst[:, 0:128]
        ones = cst[:, 128:256]

        kq_sb = cpool.tile([128, BPC, D_IN], f32)
        nc.sync.dma_start(out=kq_sb, in_=kq[:, :, :])
        wv_sb = cpool.tile([128, 4, D], f32)
        nc.sync.dma_start(out=wv_sb, in_=wv[:, :, :])
        wphi_sb = cpool.tile([128, 4, C], f32)
        nc.sync.dma_start(out=wphi_sb, in_=wphi[:, :, :])
        bvr_sb = cpool.tile([128, 4], f32)
        nc.sync.dma_start(out=bvr_sb, in_=bvr[:, :])
        bphir_sb = cpool.tile([BPC, C], f32)
        nc.sync.dma_start(out=bphir_sb, in_=bphir[:, :])

        for _rep in range(repeat):
            u_sbs = []

            for b in range(BPC):
                # Partition p holds items p*32..p*32+31 (64KB contiguous per
                # partition => efficient DMA descriptors). Sparsemax is
                # permutation-invariant and attn/x chunks stay paired, so the
                # item relabeling is harmless.
                xb = xpool.tile([128, NCHUNK, D_IN], f32, tag="xb")
                xsrc = x[b].rearrange("(p c) d -> p c d", p=128)
                for i in range(4):
                    nc.sync.dma_start(out=xb[:, 8 * i:8 * (i + 1), :],
                                      in_=xsrc[:, 8 * i:8 * (i + 1), :])

                # scores z[p, c] = x[n=c*128+p, :] . kq   (DVE dot per partition)
                z_sc = zpool.tile([128, NCHUNK], f32, tag="zsc")
                if not do_scores:
                    nc.vector.memset(z_sc, 0.01)
                for c in range(NCHUNK if do_scores else 0):
                    prod = ppool.tile([128, D_IN], f32, tag="prod")
                    nc.vector.scalar_tensor_tensor(
                        out=prod, in0=xb[:, c, :], scalar=1.0,
                        in1=kq_sb[:, b, :],
                        op0=OP.mult, op1=OP.mult,
                        accum_out=z_sc[:, c:c + 1])

                # tau0 = max(z) - 1, maintained negated+replicated [128,1]
                if not do_newton:
                    negtau = npool.tile([128, 1], f32, tag="negtau")
                    nc.vector.memset(negtau, 0.0)
                mx = npool.tile([128, 1], f32, tag="mx")
                if do_newton:
                    nc.vector.tensor_reduce(out=mx, in_=z_sc, axis=AX.X, op=OP.max)
                if do_newton:
                    ps_t = pspool.tile([1, 128], f32, tag="ps_small")
                    nc.tensor.transpose(ps_t, mx, ident)
                    vmax = npool.tile([1, 1], f32, tag="vmax")
                    nc.vector.tensor_reduce(out=vmax, in_=ps_t, axis=AX.X, op=OP.max)
                    ps_bc = pspool.tile([128, 1], f32, tag="ps_small")
                    nc.tensor.matmul(ps_bc, ones[0:1, :], vmax, start=True, stop=True)
                    negtau = npool.tile([128, 1], f32, tag="negtau")
                    nc.vector.tensor_scalar(out=negtau, in0=ps_bc,
                                            scalar1=-1.0, scalar2=1.0,
                                            op0=OP.mult, op1=OP.add)

                # Newton: tau += (S(tau)-1)/count(tau)
                for _it in range(nit if do_newton else 0):
                    SC = npool.tile([128, 2], f32, tag="SC")
                    relu_s = ppool.tile([128, NCHUNK], f32, tag="relus")
                    nc.scalar.activation(out=relu_s, in_=z_sc, func=AF.Relu,
                                         bias=negtau, scale=1.0,
                                         accum_out=SC[:, 0:1])
                    gt_s = ppool.tile([128, NCHUNK], f32, tag="gts")
                    nc.vector.tensor_scalar(out=gt_s, in0=relu_s,
                                            scalar1=0.0, scalar2=None,
                                            op0=OP.is_gt, op1=OP.add,
                                            accum_out=SC[:, 1:2])
                    ps_sc = pspool.tile([128, 2], f32, tag="ps_small")
                    nc.tensor.matmul(ps_sc, ones, SC, start=True, stop=True)
                    rcp = npool.tile([128, 1], f32, tag="rcp")
                    nc.vector.reciprocal(out=rcp, in_=ps_sc[:, 1:2])
                    delta = npool.tile([128, 1], f32, tag="delta")
                    nc.vector.scalar_tensor_tensor(
                        out=delta, in0=ps_sc[:, 0:1], scalar=-1.0, in1=rcp,
                        op0=OP.add, op1=OP.mult)
                    negtau2 = npool.tile([128, 1], f32, tag="negtau")
                    nc.vector.scalar_tensor_tensor(
                        out=negtau2, in0=delta, scalar=-1.0, in1=negtau,
                        op0=OP.mult, op1=OP.add)
                    negtau = negtau2

                attn_t = apool.tile([128, NCHUNK], f32, tag="attn")
                nc.scalar.activation(out=attn_t, in_=z_sc, func=AF.Relu,
                                     bias=negtau, scale=1.0)

                # u = attn @ x  -> [1, 512] accumulated over item chunks
                ps_u = psupool.tile([1, D_IN], f32, tag="psu")
                if do_u:
                    for c in range(NCHUNK):
                        nc.tensor.matmul(ps_u, attn_t[:, c:c + 1], xb[:, c, :],
                                         start=(c == 0), stop=(c == NCHUNK - 1))
                else:
                    nc.tensor.matmul(ps_u, attn_t[:, 0:1], xb[:, 0, :],
                                     start=True, stop=True)
                u_sb = tpool.tile([1, D_IN], f32, tag=f"usb{b}")
                nc.vector.tensor_copy(out=u_sb, in_=ps_u)
                u_sbs.append(u_sb)

            # tail: z = U@Wv + bv ; out = z@Wphi + bphi
            uT = tpool.tile([128, 4, BPC], f32, tag="uT")
            if not do_tail:
                nc.vector.memset(uT, 0.0)
            for ic in range(4 if do_tail else 0):
                for b in range(BPC):
                    ps_tt = pspool.tile([128, 1], f32, tag="ps_small")
                    nc.tensor.transpose(ps_tt,
                                        u_sbs[b][0:1, ic * 128:(ic + 1) * 128],
                                        ident[0:1, 0:1])
                    nc.vector.tensor_copy(out=uT[:, ic, b:b + 1], in_=ps_tt)

            zT = tpool.tile([128, 4, BPC], f32, tag="zT")
            if not do_tail:
                nc.vector.memset(zT, 0.0)
            for jc in range(4 if do_tail else 0):
                ps_z = pspool.tile([128, BPC], f32, tag="ps_small")
                for ic in range(4):
                    nc.tensor.matmul(ps_z, wv_sb[:, ic, jc * 128:(jc + 1) * 128],
                                     uT[:, ic, :], start=(ic == 0), stop=(ic == 3))
                nc.vector.tensor_scalar(out=zT[:, jc, :], in0=ps_z,
                                        scalar1=bvr_sb[:, jc:jc + 1], scalar2=None,
                                        op0=OP.add)

            out_sb = tpool.tile([BPC, C], f32, tag="out_sb")
            for h in range(2):
                ps_o = psupool.tile([BPC, C // 2], f32, tag="pso")
                for jc in range(4):
                    nc.tensor.matmul(ps_o, zT[:, jc, :],
                                     wphi_sb[:, jc, (C // 2) * h:(C // 2) * (h + 1)],
                                     start=(jc == 0), stop=(jc == 3))
                nc.vector.tensor_add(out_sb[:, (C // 2) * h:(C // 2) * (h + 1)],
                                     ps_o, bphir_sb[:, (C // 2) * h:(C // 2) * (h + 1)])
            nc.sync.dma_start(out=out[:, :], in_=out_sb)

    nc.compile()
    return nc


def host_prep(inputs):
    """Host-side O(B*D) prep: fold q/Wk/scale into per-batch kq vectors and
    pre-layout the small weight tensors for SBUF-friendly DMA."""
    f = lambda k: np.ascontiguousarray(np.asarray(inputs[k], dtype=np.float32))
    x_items, x_query = f("x_items"), f("x_query")
    Wq, bq, Wk = f("Wq"), f("bq"), f("Wk")
    Wv, bv, Wphi, bphi = f("Wv"), f("bv"), f("Wphi"), f("bphi")

    s = np.float32(D ** -0.5)
    Q = (x_query @ Wq + bq).astype(np.float32)            # [B, D]
    KQ = ((Q @ Wk.T) * s).astype(np.float32)              # [B, D_IN]

    wv_t = np.ascontiguousarray(Wv.reshape(4, 128, D).transpose(1, 0, 2))
    wphi_t = np.ascontiguousarray(Wphi.reshape(4, 128, C).transpose(1, 0, 2))
    bvr = np.ascontiguousarray(bv.reshape(4, 128).T)
    bphir = np.ascontiguousarray(np.broadcast_to(bphi, (BPC, C)))
    consts = np.concatenate([np.eye(128, dtype=np.float32),
                             np.ones((128, 128), np.float32)], axis=1)
    consts = np.ascontiguousarray(consts)

    in_maps = []
    for core in range(NCORES):
        sl = slice(core * BPC, (core + 1) * BPC)
        kq_c = np.ascontiguousarray(
            np.broadcast_to(KQ[sl][:, None, :], (BPC, 128, D_IN))
            .transpose(1, 0, 2))                          # [128, BPC, D_IN]
        in_maps.append({
            "x": np.ascontiguousarray(x_items[sl]),
            "kq": kq_c,
            "wv": wv_t,
            "wphi": wphi_t,
            "bvr": bvr,
            "bphir": bphir,
            "consts": consts,
        })
    return in_maps


def kernel(**inputs):
    from concourse.bass_utils import run_bass_kernel_spmd

    if "nc" not in _CACHE:
        _CACHE["nc"] = build_bass()
    nc = _CACHE["nc"]

    in_maps = host_prep(inputs)
    res = run_bass_kernel_spmd(nc, in_maps, list(range(NCORES)))
    return np.concatenate([res.results[c]["out"] for c in range(NCORES)],
                          axis=0).astype(np.float32)


# revision 11
# speedup vs baseline: 306.4229x; 1.8601x over previous
"""Trainium2 Bass kernel for nn_MaxRetrievalModel (sparse attention retrieval).

Math: the reference computes
    q = x_query[..,None]@Wq + bq                 [B,1,D]
    k = x_items@Wk + bk ; v = x_items@Wv + bv    [B,N,D]
    scores = (q.k)/sqrt(D) ; attn = sparsemax(scores)
    out = (attn@v)[:,0]@Wphi + bphi              [B,C]

Two exact algebraic identities collapse the 137-GFLOP K/V projections:
  * sparsemax is shift-invariant, so the bk term (constant over items)
    vanishes, and scores = x_items @ (s*Wk@q[b]) — a matvec per batch.
  * sparsemax output sums to exactly 1, so attn@v = (attn@x_items)@Wv + bv.
The sparsemax threshold tau solves sum(relu(z - tau)) = 1; Newton's method
on that piecewise-linear equation (monotone from tau0 = max(z) - 1)
converges exactly in <= ~9 steps for this regime, avoiding any sort.

Device work per core (4 batches): stream x shard (33.5 MB) once into SBUF,
per-partition dot products on DVE for scores, Newton on ACT/DVE/PE,
attn@x on PE, then the small Wv/Wphi tail on PE.

Sharding: data-parallel over batch B=32 across 8 cores (4 batches/core).
"""

import sys

if "/opt/trn_rl_repo" not in sys.path:
    sys.path.insert(0, "/opt/trn_rl_repo")

import numpy as np

B, N, D_IN, D, C = 32, 4096, 512, 512, 1000
NCORES = 8
BPC = B // NCORES          # batches per core
NCHUNK = N // 128          # 32 item chunks of 128
NIT = 11                   # Newton iterations (converges in <= 9 on this regime)

_CACHE = {}


def build_bass(repeat=1, nit=NIT, do_scores=True, do_newton=True, do_u=True,
               do_tail=True):
    """Build (and bacc-compile) the per-core Bass module.

    repeat > 1 replays the whole body (differential hardware timing);
    the do_* flags ablate stages for cost attribution. Correct output
    requires all stages on.
    """
    import concourse.bacc as bacc
    import concourse.tile as tile
    from concourse import mybir
    from contextlib import ExitStack

    f32 = mybir.dt.float32
    f32r = mybir.dt.float32r     # TF32-like fast PE mode: 1 cyc/row vs 4 for fp32
    r = lambda ap: ap.bitcast(f32r)
    AF = mybir.ActivationFunctionType
    OP = mybir.AluOpType
    AX = mybir.AxisListType

    nc = bacc.Bacc("TRN2", target_bir_lowering=False, debug=False,
                   num_devices=NCORES)

    x = nc.declare_dram_parameter("x", [BPC, N, D_IN], f32, isOutput=False)
    kq = nc.declare_dram_parameter("kq", [128, BPC, D_IN], f32, isOutput=False)
    wv = nc.declare_dram_parameter("wv", [128, 4, D], f32, isOutput=False)
    wphi = nc.declare_dram_parameter("wphi", [128, 4, C], f32, isOutput=False)
    bvr = nc.declare_dram_parameter("bvr", [128, 4], f32, isOutput=False)
    bphir = nc.declare_dram_parameter("bphir", [BPC, C], f32, isOutput=False)
    consts = nc.declare_dram_parameter("consts", [128, 256], f32, isOutput=False)
    out = nc.declare_dram_parameter("out", [BPC, C], f32, isOutput=True)

    with tile.TileContext(nc) as tc, ExitStack() as ctx:
        cpool = ctx.enter_context(tc.tile_pool(name="consts", bufs=1))
        xpool = ctx.enter_context(tc.tile_pool(name="xb", bufs=2))
        zpool = ctx.enter_context(tc.tile_pool(name="zsc", bufs=2))
        ppool = ctx.enter_context(tc.tile_pool(name="prod", bufs=2))
        npool = ctx.enter_context(tc.tile_pool(name="newton", bufs=4))
        apool = ctx.enter_context(tc.tile_pool(name="attn", bufs=2))
        tpool = ctx.enter_context(tc.tile_pool(name="tail", bufs=2))
        pspool = ctx.enter_context(tc.tile_pool(name="ps", bufs=3, space="PSUM"))
        psupool = ctx.enter_context(tc.tile_pool(name="psu", bufs=2, space="PSUM"))

        cst = cpool.tile([128, 256], f32)
        nc.sync.dma_start(out=cst, in_=consts[:, :])
        ident = cst[:, 0:128]
        ones = cst[:, 128:256]

        kq_sb = cpool.tile([128, BPC, D_IN], f32)
        nc.sync.dma_start(out=kq_sb, in_=kq[:, :, :])
        wv_sb = cpool.tile([128, 4, D], f32)
        nc.sync.dma_start(out=r(wv_sb), in_=r(wv[:, :, :]))
        wphi_sb = cpool.tile([128, 4, C], f32)
        nc.sync.dma_start(out=r(wphi_sb), in_=r(wphi[:, :, :]))
        bvr_sb = cpool.tile([128, 4], f32)
        nc.sync.dma_start(out=bvr_sb, in_=bvr[:, :])
        bphir_sb = cpool.tile([BPC, C], f32)
        nc.sync.dma_start(out=bphir_sb, in_=bphir[:, :])

        for _rep in range(repeat):
            u_sbs = []

            for b in range(BPC):
                # Partition p holds items p*32..p*32+31 (64KB contiguous per
                # partition => efficient DMA descriptors). Sparsemax is
                # permutation-invariant and attn/x chunks stay paired, so the
                # item relabeling is harmless.
                xb = xpool.tile([128, NCHUNK, D_IN], f32, tag="xb")
                xsrc = x[b].rearrange("(p c) d -> p c d", p=128)
                for i in range(4):
                    nc.sync.dma_start(out=r(xb[:, 8 * i:8 * (i + 1), :]),
                                      in_=r(xsrc[:, 8 * i:8 * (i + 1), :]))

                # scores z[p, c] = x[n=c*128+p, :] . kq   (DVE dot per partition)
                z_sc = zpool.tile([128, NCHUNK], f32, tag="zsc")
                if not do_scores:
                    nc.vector.memset(z_sc, 0.01)
                for c in range(NCHUNK if do_scores else 0):
                    prod = ppool.tile([128, D_IN], f32, tag="prod")
                    nc.vector.scalar_tensor_tensor(
                        out=prod, in0=xb[:, c, :], scalar=1.0,
                        in1=kq_sb[:, b, :],
                        op0=OP.mult, op1=OP.mult,
                        accum_out=z_sc[:, c:c + 1])

                # tau0 = max(z) - 1, maintained negated+replicated [128,1]
                if not do_newton:
                    negtau = npool.tile([128, 1], f32, tag="negtau")
                    nc.vector.memset(negtau, 0.0)
                mx = npool.tile([128, 1], f32, tag="mx")
                if do_newton:
                    nc.vector.tensor_reduce(out=mx, in_=z_sc, axis=AX.X, op=OP.max)
                if do_newton:
                    ps_t = pspool.tile([1, 128], f32, tag="ps_small")
                    nc.tensor.transpose(ps_t, mx, ident)
                    vmax = npool.tile([1, 1], f32, tag="vmax")
                    nc.vector.tensor_reduce(out=vmax, in_=ps_t, axis=AX.X, op=OP.max)
                    ps_bc = pspool.tile([128, 1], f32, tag="ps_small")
                    nc.tensor.matmul(ps_bc, ones[0:1, :], vmax, start=True, stop=True)
                    negtau = npool.tile([128, 1], f32, tag="negtau")
                    nc.vector.tensor_scalar(out=negtau, in0=ps_bc,
                                            scalar1=-1.0, scalar2=1.0,
                                            op0=OP.mult, op1=OP.add)

                # Newton: tau += (S(tau)-1)/count(tau)
                for _it in range(nit if do_newton else 0):
                    SC = npool.tile([128, 2], f32, tag="SC")
                    relu_s = ppool.tile([128, NCHUNK], f32, tag="relus")
                    nc.scalar.activation(out=relu_s, in_=z_sc, func=AF.Relu,
                                         bias=negtau, scale=1.0,
                                         accum_out=SC[:, 0:1])
                    gt_s = ppool.tile([128, NCHUNK], f32, tag="gts")
                    nc.vector.tensor_scalar(out=gt_s, in0=relu_s,
                                            scalar1=0.0, scalar2=None,
                                            op0=OP.is_gt, op1=OP.add,
                                            accum_out=SC[:, 1:2])
                    ps_sc = pspool.tile([128, 2], f32, tag="ps_small")
                    nc.tensor.matmul(ps_sc, ones, SC, start=True, stop=True)
                    rcp = npool.tile([128, 1], f32, tag="rcp")
                    nc.vector.reciprocal(out=rcp, in_=ps_sc[:, 1:2])
                    delta = npool.tile([128, 1], f32, tag="delta")
                    nc.vector.scalar_tensor_tensor(
                        out=delta, in0=ps_sc[:, 0:1], scalar=-1.0, in1=rcp,
                        op0=OP.add, op1=OP.mult)
                    negtau2 = npool.tile([128, 1], f32, tag="negtau")
                    nc.vector.scalar_tensor_tensor(
                        out=negtau2, in0=delta, scalar=-1.0, in1=negtau,
                        op0=OP.mult, op1=OP.add)
                    negtau = negtau2

                attn_t = apool.tile([128, NCHUNK], f32, tag="attn")
                nc.scalar.activation(out=r(attn_t), in_=z_sc, func=AF.Relu,
                                     bias=negtau, scale=1.0)

                # u = attn @ x  -> [1, 512] accumulated over item chunks
                ps_u = psupool.tile([1, D_IN], f32, tag="psu")
                if do_u:
                    for c in range(NCHUNK):
                        nc.tensor.matmul(ps_u, r(attn_t[:, c:c + 1]),
                                         r(xb[:, c, :]),
                                         start=(c == 0), stop=(c == NCHUNK - 1))
                else:
                    nc.tensor.matmul(ps_u, r(attn_t[:, 0:1]), r(xb[:, 0, :]),
                                     start=True, stop=True)
                u_sb = tpool.tile([1, D_IN], f32, tag=f"usb{b}")
                nc.vector.tensor_copy(out=u_sb, in_=ps_u)
                u_sbs.append(u_sb)

            # tail: z = U@Wv + bv ; out = z@Wphi + bphi
            uT = tpool.tile([128, 4, BPC], f32, tag="uT")
            if not do_tail:
                nc.vector.memset(uT, 0.0)
            for ic in range(4 if do_tail else 0):
                for b in range(BPC):
                    ps_tt = pspool.tile([128, 1], f32, tag="ps_small")
                    nc.tensor.transpose(ps_tt,
                                        u_sbs[b][0:1, ic * 128:(ic + 1) * 128],
                                        ident[0:1, 0:1])
                    nc.vector.tensor_copy(out=r(uT[:, ic, b:b + 1]), in_=ps_tt)

            zT = tpool.tile([128, 4, BPC], f32, tag="zT")
            if not do_tail:
                nc.vector.memset(zT, 0.0)
            for jc in range(4 if do_tail else 0):
                ps_z = pspool.tile([128, BPC], f32, tag="ps_small")
                for ic in range(4):
                    nc.tensor.matmul(ps_z,
                                     r(wv_sb[:, ic, jc * 128:(jc + 1) * 128]),
                                     r(uT[:, ic, :]),
                                     start=(ic == 0), stop=(ic == 3))
                nc.vector.tensor_scalar(out=r(zT[:, jc, :]), in0=ps_z,
                                        scalar1=bvr_sb[:, jc:jc + 1], scalar2=None,
                                        op0=OP.add)

            out_sb = tpool.tile([BPC, C], f32, tag="out_sb")
            for h in range(2):
                ps_o = psupool.tile([BPC, C // 2], f32, tag="pso")
                for jc in range(4):
                    nc.tensor.matmul(ps_o, r(zT[:, jc, :]),
                                     r(wphi_sb[:, jc, (C // 2) * h:(C // 2) * (h + 1)]),
                                     start=(jc == 0), stop=(jc == 3))
                nc.vector.tensor_add(out_sb[:, (C // 2) * h:(C // 2) * (h + 1)],
                                     ps_o, bphir_sb[:, (C // 2) * h:(C // 2) * (h + 1)])
            nc.sync.dma_start(out=out[:, :], in_=out_sb)

    nc.compile()
    return nc


def host_prep(inputs):
    """Host-side O(B*D) prep: fold q/Wk/scale into per-batch kq vectors and
    pre-layout the small weight tensors for SBUF-friendly DMA."""
    f = lambda k: np.ascontiguousarray(np.asarray(inputs[k], dtype=np.float32))
    x_items, x_query = f("x_items"), f("x_query")
    Wq, bq, Wk = f("Wq"), f("bq"), f("Wk")
    Wv, bv, Wphi, bphi = f("Wv"), f("bv"), f("Wphi"), f("bphi")

    s = np.float32(D ** -0.5)
    Q = (x_query @ Wq + bq).astype(np.float32)            # [B, D]
    KQ = ((Q @ Wk.T) * s).astype(np.float32)              # [B, D_IN]

    wv_t = np.ascontiguousarray(Wv.reshape(4, 128, D).transpose(1, 0, 2))
    wphi_t = np.ascontiguousarray(Wphi.reshape(4, 128, C).transpose(1, 0, 2))
    bvr = np.ascontiguousarray(bv.reshape(4, 128).T)
    bphir = np.ascontiguousarray(np.broadcast_to(bphi, (BPC, C)))
    consts = np.concatenate([np.eye(128, dtype=np.float32),
                             np.ones((128, 128), np.float32)], axis=1)
    consts = np.ascontiguousarray(consts)

    in_maps = []
    for core in range(NCORES):
        sl = slice(core * BPC, (core + 1) * BPC)
        kq_c = np.ascontiguousarray(
            np.broadcast_to(KQ[sl][:, None, :], (BPC, 128, D_IN))
            .transpose(1, 0, 2))                          # [128, BPC, D_IN]
        in_maps.append({
            "x": np.ascontiguousarray(x_items[sl]),
            "kq": kq_c,
            "wv": wv_t,
            "wphi": wphi_t,
            "bvr": bvr,
            "bphir": bphir,
            "consts": consts,
        })
    return in_maps


def kernel(**inputs):
    from concourse.bass_utils import run_bass_kernel_spmd

    if "nc" not in _CACHE:
        _CACHE["nc"] = build_bass()
    nc = _CACHE["nc"]

    in_maps = host_prep(inputs)
    res = run_bass_kernel_spmd(nc, in_maps, list(range(NCORES)))
    return np.concatenate([res.results[c]["out"] for c in range(NCORES)],
                          axis=0).astype(np.float32)
